# revision 44
# baseline (speedup 1.0000x reference)
"""Multi-head attention (B=2, S=2048, D=1024, H=16) on 8 TRN2 NeuronCores.

Sharding: data-parallel over batch (2) x tensor-parallel over head groups (4).
Core c handles batch c//4, heads 4*(c%4) .. 4*(c%4)+3 (256 projection dims).
Each core computes its partial output projection; the host sums the 4 partials
per batch and adds the (bv @ wo.T + bo) constant, which is exact because
softmax weights sum to 1.

Device layouts (per core):
  QT/KT  [128, 2, 2048] bf16 : partition p + 128*m = local proj dim, free = seq
  V_sb   [128, 16, 4, 65] bf16: [k-pos within tile, k-tile, head, dk + ones col]
  scores computed transposed: ST[k, q] = K'h @ Q'h^T, exp mostly on ScalarE
  with a fraction offloaded to DVE via an int16 Schraudolph bit-trick,
  PV: attnT += [Vh|1]^T @ expST  (ones column yields the softmax denominator)
  out-proj: out[s, dout] = attnT^T @ woT, partial, bf16 to DRAM.

TRN2's tensor engine clocks down (2.4 -> 1.2 GHz) whenever it idles and needs
~3us of continuous execution to ramp back, so the emission is built around
never letting the PE queue drain: projections are dripped between attention
kt-steps, warm-up matmuls run while the first DMAs land, and heater matmuls
bridge the final normalization latency.
"""

import sys

sys.path.insert(0, "/opt/trn_rl_repo")

import numpy as np
import ml_dtypes

BF16 = ml_dtypes.bfloat16

B, S, D = 2, 2048, 1024
H, DK = 16, 64
N_CORES = 8
GROUPS = 4  # head groups (tensor-parallel)
DL = D // GROUPS  # 256 local projection dims per core
SCALE = 1.0 / np.sqrt(np.sqrt(float(DK)))  # fold 1/sqrt(dk) half into Q, half into K

# kt indices per (hp, qb) whose exp runs on DVE (Schraudolph) instead of ScalarE
DVE_KTS = (2, 5, 8, 11, 14)
# int16 Schraudolph constants: bits = s*128*log2(e) + 16256 + C (+0.5 trunc guard)
SCH_A = 184.66452
SCH_B = 16256.0 - 7.5 + 0.5

import os
V_HEAT = os.environ.get("V_HEAT", "1") == "1"
V_GPMUL = os.environ.get("V_GPMUL", "1") == "1"
V_PRE = os.environ.get("V_PRE", "0") == "1"
V_WO2 = os.environ.get("V_WO2", "0") == "1"

_cache: dict = {}


def _build():
    import concourse.mybir as mybir
    import concourse.tile as tile
    from concourse import bacc

    dt = mybir.dt
    f32, bf16 = dt.float32, dt.bfloat16

    nc = bacc.Bacc("TRN2", target_bir_lowering=False, debug=False,
                   num_devices=N_CORES)

    # prepacked inputs: [sb, p, kt, s] so each DMA is contiguous per partition
    xq_pk = nc.dram_tensor("xq_pk", [4, 128, 8, 512], bf16, kind="ExternalInput").ap()
    xk_pk = nc.dram_tensor("xk_pk", [4, 128, 8, 512], bf16, kind="ExternalInput").ap()
    xv_pk = nc.dram_tensor("xv_pk", [4, 128, 8, 512], bf16, kind="ExternalInput").ap()
    wq_pk = nc.dram_tensor("wq_pk", [128, 8, DL], bf16, kind="ExternalInput").ap()
    wk_pk = nc.dram_tensor("wk_pk", [128, 8, DL], bf16, kind="ExternalInput").ap()
    wv_pk = nc.dram_tensor("wv_pk", [128, 8, DL], bf16, kind="ExternalInput").ap()
    wo_pk = nc.dram_tensor("wo_pk", [128, 2, D], bf16, kind="ExternalInput").ap()
    bqk = nc.dram_tensor("bqk", [2, DL], f32, kind="ExternalInput").ap()
    out = nc.dram_tensor("out", [S, D], bf16, kind="ExternalOutput").ap()

    EXPF = mybir.ActivationFunctionType.Exp
    COPYF = mybir.ActivationFunctionType.Copy
    MULT, ADD = mybir.AluOpType.mult, mybir.AluOpType.add

    with tile.TileContext(nc) as tc:
        with (
            tc.tile_pool(name="res", bufs=1) as res,
            tc.tile_pool(name="wts", bufs=1) as wts,
            tc.tile_pool(name="xin", bufs=3) as xin,
            tc.tile_pool(name="expp", bufs=8) as expp,
            tc.tile_pool(name="nrm", bufs=4) as nrm,
            tc.tile_pool(name="drm", bufs=2, space="DRAM") as drm,
            tc.tile_pool(name="ps_proj", bufs=2, space="PSUM") as ps_proj,
            tc.tile_pool(name="ps_at", bufs=2, space="PSUM") as ps_at,
            tc.tile_pool(name="ps_st", bufs=2, space="PSUM") as ps_st,
        ):
            # ---- resident tensors ----
            QT = [res.tile([128, S], bf16, name=f"QT{m}", tag=f"QT{m}")
                  for m in range(2)]
            KT = [res.tile([128, S], bf16, name=f"KT{m}", tag=f"KT{m}")
                  for m in range(2)]
            Vsb = res.tile([128, 16, 4, DK + 1], bf16)
            ATT = [res.tile([128, S], bf16, name=f"ATT{m}", tag=f"ATT{m}")
                   for m in range(2)]

            wq_sb = wts.tile([128, 8, DL], bf16, tag="wq")
            wk_sb = wts.tile([128, 8, DL], bf16, tag="wk")
            wv_sb = wts.tile([128, 8, DL], bf16, tag="wv")
            wo_sb = wts.tile([128, 2, D], bf16, tag="wo")
            wo2_sb = wts.tile([64, D], bf16, tag="wo2")
            b_sb = wts.tile([128, 2, 2], f32, tag="b")  # [p, proj(q/k), m]
            ones_sb = wts.tile([1, 64], bf16, tag="ones")
            ones64f = wts.tile([65, 64], f32, tag="ones64")
            ones64b = wts.tile([65, 64], bf16, tag="ones64b")
            wrm_sb = wts.tile([1, 512], bf16, tag="wrm")

            # resident x tiles for Q/K; sb=0 is split in half so the first
            # projection's k-accumulation can start on the first 4 kt the
            # moment that half lands (fine-grained DMA deps)
            XQ = [None] + [res.tile([128, 8, 512], bf16, name=f"XQ{sb}",
                                    tag=f"XQ{sb}") for sb in range(1, 4)]
            XK = [None] + [res.tile([128, 8, 512], bf16, name=f"XK{sb}",
                                    tag=f"XK{sb}") for sb in range(1, 4)]
            XQ0 = [res.tile([128, 4, 512], bf16, name=f"XQ0{h}", tag=f"XQ0{h}")
                   for h in range(2)]
            XK0 = [res.tile([128, 4, 512], bf16, name=f"XK0{h}", tag=f"XK0{h}")
                   for h in range(2)]

            def xslice(pj, sb, kt):
                """rhs AP for (q/k) seq-block sb, contraction tile kt."""
                if sb == 0:
                    t = (XQ0 if pj == 0 else XK0)[kt // 4]
                    return t[:, kt % 4, :]
                t = (XQ if pj == 0 else XK)[sb]
                return t[:, kt, :]

            nc.vector.memset(Vsb[:, :, :, DK], 1.0)
            nc.vector.memset(ones_sb[:, :], 1.0)
            nc.vector.memset(ones64f[64:65, :], 1.0)
            nc.vector.memset(ones64b[64:65, :], 1.0)
            nc.vector.memset(wrm_sb[:, :], 1.0)

            # preload the Exp activation table (content unused)
            warm = nrm.tile([1, 1], f32, tag="warm")
            nc.scalar.activation(warm[:, :], wrm_sb[0:1, 0:1], EXPF)

            def heater(n, tag, w=512):
                """n dependency-free matmuls that keep the tensor engine's
                p-state at max through a gap (content discarded). w=128 gives
                ~53ns grains for fine-grained bridging."""
                if not V_HEAT:
                    return
                hps = ps_st.tile([128, 2, 512], f32, tag="st",
                                 name=f"heat{tag}")
                for i in range(n):
                    nc.tensor.matmul(hps[0:64, 0, 0:w], lhsT=ones_sb[:, :],
                                     rhs=wrm_sb[:, 0:w], start=True,
                                     stop=True)

            def qk_proj_fillers(m, pj, sb):
                """Two small pure-matmul closures (x is resident): first half
                of the k-accumulation, then second half + eviction. Dripped
                into attention's kt loop so they never hog PE long enough to
                starve ScalarE."""
                wsb, dst = [(wq_sb, QT), (wk_sb, KT)][pj]
                state = {}

                def part1():
                    ps = ps_proj.tile([128, 512], f32, tag="proj",
                                      name=f"psp{m}{pj}{sb}")
                    for kt in range(4):
                        nc.tensor.matmul(
                            ps[:, :],
                            lhsT=wsb[:, kt, m * 128:(m + 1) * 128],
                            rhs=xslice(pj, sb, kt),
                            start=(kt == 0), stop=False)
                    state["ps"] = ps

                def part2():
                    ps = state["ps"]
                    for kt in range(4, 8):
                        nc.tensor.matmul(
                            ps[:, :],
                            lhsT=wsb[:, kt, m * 128:(m + 1) * 128],
                            rhs=xslice(pj, sb, kt),
                            start=False, stop=(kt == 7))
                    nc.vector.tensor_scalar_add(
                        dst[m][:, sb * 512:(sb + 1) * 512],
                        ps[:, :], b_sb[:, pj, m:m + 1])

                return [part1, part2]

            def emit_qk_proj(m, pj, sbs):
                for sb in sbs:
                    for f in qk_proj_fillers(m, pj, sb):
                        f()

            vxb = {}

            def v_dma_filler(stq, eng=None):
                def f():
                    e = eng or nc.gpsimd
                    xb = xin.tile([128, 8, 512], bf16, tag="xblk",
                                  name=f"xbv{stq}")
                    e.dma_start(xb[:, 0:4, :], xv_pk[stq, :, 0:4, :])
                    e.dma_start(xb[:, 4:8, :], xv_pk[stq, :, 4:8, :])
                    vxb[stq] = xb
                return f

            def v_st_filler(st):
                def f():
                    xb = vxb[st // 4]
                    sts = st % 4
                    ps = ps_proj.tile([128, 4, DK], f32, tag="proj",
                                      name=f"psv{st}")
                    for kt in range(8):
                        nc.tensor.matmul(
                            ps[:, :, :],
                            lhsT=xb[:, kt, sts * 128:(sts + 1) * 128],
                            rhs=wv_sb[:, kt, :],
                            start=(kt == 0), stop=(kt == 7))
                    nc.vector.tensor_copy(Vsb[:, st, :, 0:DK], ps[:, :, :])
                return f

            def alloc_pa(hp, qb):
                return [ps_at.tile([65, 512], f32, tag="at",
                                   name=f"at{hp}{qb}{i}") for i in range(2)]

            def emit_attn_block(hp, qb, pa, sched=None, dve=True,
                                pre=None):
                """Software-pipelined 16-kt attention block: scores for kt+2
                are emitted before the exp-gated PV of kt so the in-order PE
                queue never idles behind a semaphore wait; drip fillers run
                between the lookahead scores and the PV. sched maps iter
                (-2..15) -> list of filler closures. pre carries exp tiles
                for kt 0/1 already emitted inside the PREVIOUS block's last
                iterations (cross-block lookahead), so this block's kt=2
                score never waits on exp(0) latency at the boundary."""
                qs = slice(qb * 512, (qb + 1) * 512)
                sched = sched or {}
                ex_tiles = pre if pre is not None else {}

                def sc_exp(kt):
                    # both heads' score tiles back-to-back: disjoint row
                    # groups (partitions 0-63 / 64-127) co-stream on PE
                    st_ps = ps_st.tile([128, 2, 512], f32, tag="st")
                    for hh in range(2):
                        lo, hi = hh * 64, hh * 64 + 64
                        nc.tensor.matmul(
                            st_ps[:, hh, :],
                            lhsT=KT[hp][lo:hi, kt * 128:(kt + 1) * 128],
                            rhs=QT[hp][lo:hi, qs],
                            start=True, stop=True)
                    ex = expp.tile([128, 2, 512], bf16, tag="exp")
                    if dve and kt in DVE_KTS:
                        # exp via int16 Schraudolph on DVE (result read as
                        # bf16); relieves the ScalarE exp bottleneck
                        nc.vector.tensor_scalar(
                            ex[:, :, :].bitcast(dt.int16), st_ps[:, :, :],
                            SCH_A, SCH_B, MULT, ADD)
                    else:
                        nc.scalar.activation(ex[:, :, :], st_ps[:, :, :], EXPF)
                    ex_tiles[kt] = ex

                def run(it):
                    for f in sched.get(it, ()):
                        f()

                run(-2)
                if 0 not in ex_tiles:
                    sc_exp(0)
                run(-1)
                if 1 not in ex_tiles:
                    sc_exp(1)
                # PV in kt-pairs: two consecutive matmuls of the SAME
                # accumulation group back-to-back overlap their stationary
                # loads with streaming (like the V-proj chains do), unlike
                # the alternating pa[0]/pa[1] order
                for kt2 in range(0, 16, 2):
                    if kt2 + 2 < 16:
                        sc_exp(kt2 + 2)
                    run(kt2)
                    if kt2 + 3 < 16:
                        sc_exp(kt2 + 3)
                    run(kt2 + 1)
                    exa = ex_tiles.pop(kt2)
                    exb = ex_tiles.pop(kt2 + 1)
                    for hh in range(2):
                        nc.tensor.matmul(
                            pa[hh][:, :],
                            lhsT=Vsb[:, kt2, 2 * hp + hh, :],
                            rhs=exa[:, hh, :],
                            start=(kt2 == 0), stop=False,
                            skip_group_check=True)
                        nc.tensor.matmul(
                            pa[hh][:, :],
                            lhsT=Vsb[:, kt2 + 1, 2 * hp + hh, :],
                            rhs=exb[:, hh, :],
                            start=False, stop=(kt2 + 1 == 15),
                            skip_group_check=True)

            def make_pre(hp, qb, dve=True):
                """Closures emitting block (hp,qb)'s kt 0/1 score+exp inside
                the previous block's sched[14]/[15] slots. The scores land in
                ps_proj tiles (free around block boundaries) rather than the
                2-deep ps_st ring, so the lookahead genuinely deepens instead
                of stealing the ring slots kt14/15 still need."""
                qs = slice(qb * 512, (qb + 1) * 512)
                store = {}

                def one(kt):
                    def f():
                        ph = [ps_proj.tile([128, 512], f32, tag="proj",
                                           name=f"pre{hp}{qb}{kt}{hh}")
                              for hh in range(2)]
                        for hh in range(2):
                            lo, hi = hh * 64, hh * 64 + 64
                            nc.tensor.matmul(
                                ph[hh][:, :],
                                lhsT=KT[hp][lo:hi, kt * 128:(kt + 1) * 128],
                                rhs=QT[hp][lo:hi, qs],
                                start=True, stop=True)
                        ex = expp.tile([128, 2, 512], bf16, tag="exp",
                                       name=f"prx{hp}{qb}{kt}")
                        for hh in range(2):
                            nc.scalar.activation(ex[:, hh, :], ph[hh][:, :],
                                                 EXPF)
                        store[kt] = ex
                    return f

                return store, [one(0), one(1)]

            def norm_copies(pa, store):
                """Phase 1 (next block's iter-0 slot): evict both attnT PSUM
                tiles to SBUF so the ps_at ring frees for the next block,
                and pre-cast the denominator rows so phase 2's broadcast
                matmuls fire without waiting on the DVE queue."""
                def f():
                    for hh in range(2):
                        asb = nrm.tile([65, 512], f32, tag="asb",
                                       name=f"asb{hh}")
                        nc.vector.tensor_copy(asb[:, :], pa[hh][:, :])
                        dn = nrm.tile([65, 512], bf16, tag="dn",
                                      name=f"dn{hh}")
                        with nc.allow_low_precision(reason="bf16 denom"):
                            nc.vector.tensor_copy(dn[64:65, :],
                                                  pa[hh][64:65, :])
                        store[hh] = asb
                        store[2 + hh] = dn
                return f

            def norm_finish(hp, qb, store):
                """Phase 2 (iter-2 slot, so its bcd PSUM ring slots clear
                before the drained out-proj fillers alloc): bf16-cast the
                denominator row, broadcast it down partitions 0..63 with a
                one-row matmul, invert with the fast approx reciprocal
                (base-0 only, hence broadcast-first), multiply on the idle
                GPSIMD engine."""
                def f():
                    qs = slice(qb * 512, (qb + 1) * 512)
                    rbs = {}
                    for hh in range(2):
                        dn = store[2 + hh]
                        bcd = ps_proj.tile([64, 512], f32, tag="proj",
                                           name=f"bcd{hp}{qb}{hh}")
                        nc.tensor.matmul(bcd[:, :], lhsT=ones64b[64:65, :],
                                         rhs=dn[64:65, :], start=True,
                                         stop=True)
                        rb = nrm.tile([64, 512], f32, tag="rbf",
                                      name=f"rb{hh}")
                        nc.vector.reciprocal_approx_fast(rb[:, :], bcd[:, :])
                        rbs[hh] = rb
                    eng = nc.gpsimd if V_GPMUL else nc.vector
                    eng.tensor_mul(ATT[hp][0:64, qs],
                                   store[0][0:64, :], rbs[0][:, :])
                    tmp = nrm.tile([64, 512], bf16, tag="tmp")
                    eng.tensor_mul(tmp[:, :], store[1][0:64, :], rbs[1][:, :])
                    nc.sync.dma_start(ATT[hp][64:128, qs], tmp[:, :])
                return f

            def emit_attn_norm_fast(hp, qb, pa, reserve=()):
                """Latency-lean endgame norm, fully serialized per head so
                hh1 (whose result must additionally hop to ATT partitions
                64..127 via DMA) completes first: bf16-cast denominator,
                one-row broadcast matmul, fast approx reciprocal (base-0
                only), multiply. Heaters keep the PE p-state at max through
                the DVE latency."""
                qs = slice(qb * 512, (qb + 1) * 512)
                for hh in (1, 0):
                    asb = nrm.tile([65, 512], f32, tag="asb",
                                   name=f"asbf{hh}")
                    nc.vector.tensor_copy(asb[:, :], pa[hh][:, :])
                    dn = nrm.tile([65, 512], bf16, tag="dn", name=f"dnf{hh}")
                    with nc.allow_low_precision(reason="bf16 denom"):
                        nc.vector.tensor_copy(dn[64:65, :], pa[hh][64:65, :])
                    if hh == 1:
                        for f in reserve:
                            f()
                    bcd = ps_proj.tile([64, 512], f32, tag="proj",
                                       name=f"bcdf{hh}")
                    nc.tensor.matmul(bcd[:, :], lhsT=ones64b[64:65, :],
                                     rhs=dn[64:65, :], start=True,
                                     stop=True)
                    rb = nrm.tile([64, 512], f32, tag="rbf",
                                  name=f"rbf{hh}")
                    nc.vector.reciprocal_approx_fast(rb[:, :], bcd[:, :])
                    if hh == 1:
                        tmp = nrm.tile([64, 512], bf16, tag="tmp")
                        nc.vector.tensor_mul(tmp[:, :], asb[0:64, :],
                                             rb[:, :])
                        endgame_tmp["t"] = tmp
                        if not V_WO2:
                            nc.sync.dma_start(ATT[hp][64:128, qs],
                                              tmp[:, :])
                            heater(18, "t2", w=128)
                        else:
                            heater(4, "t2", w=128)
                    else:
                        nc.vector.tensor_mul(ATT[hp][0:64, qs],
                                             asb[0:64, :], rb[:, :])

            endgame_tmp = {}

            def out_proj_filler(st, db, tail=False):
                def f():
                    ps = ps_proj.tile([128, 512], f32, tag="proj",
                                      name=f"pso{st}{db}")
                    dbs = slice(db * 512, (db + 1) * 512)
                    sts = slice(st * 128, (st + 1) * 128)
                    if tail and V_WO2:
                        # last block: hh1's normalized attn is read straight
                        # from the norm's tmp tile (base 0), so the ~2.5us
                        # SBUF-partition-hop DMA never gates these matmuls
                        tmp = endgame_tmp["t"]
                        qs2 = slice((st - 12) * 128, (st - 11) * 128)
                        nc.tensor.matmul(ps[:, :], lhsT=ATT[0][:, sts],
                                         rhs=wo_sb[:, 0, dbs],
                                         start=True, stop=False)
                        nc.tensor.matmul(ps[:, :], lhsT=ATT[1][0:64, sts],
                                         rhs=wo_sb[0:64, 1, dbs],
                                         start=False, stop=False)
                        nc.tensor.matmul(ps[:, :], lhsT=tmp[:, qs2],
                                         rhs=wo2_sb[:, dbs],
                                         start=False, stop=True)
                    else:
                        for m in range(2):
                            nc.tensor.matmul(
                                ps[:, :],
                                lhsT=ATT[m][:, sts],
                                rhs=wo_sb[:, m, dbs],
                                start=(m == 0), stop=(m == 1))
                    osb = nrm.tile([128, 512], bf16, tag="osb")
                    # at the endgame both ScalarE and DVE are idle: alternate
                    # so the final 8 casts don't serialize on one engine, and
                    # rotate the final DMAs across three DGE queues so their
                    # ~0.55us trigger setups don't serialize on the sync
                    # sequencer (that queueing was ~4us of pure tail)
                    if tail and (st + db) % 2 == 0:
                        nc.scalar.activation(osb[:, :], ps[:, :], COPYF)
                    else:
                        nc.vector.tensor_copy(osb[:, :], ps[:, :])
                    nc.sync.dma_start(
                        out[st * 128:(st + 1) * 128, db * 512:(db + 1) * 512],
                        osb[:, :])
                return f

            # ---- emission: flash-style streaming. K/V/Q chunks feed
            # attention's k-tile pipeline incrementally; A1 and the output
            # projection gap-fill PE while ScalarE (exp) runs flat out ----
            # The input phase is pure HBM-bandwidth-bound (~13.5MB), so all
            # transfers are issued on ONE queue in exact need-order: the
            # critical prefix (wk/xk0/wq/xq0/biases/wv) never shares
            # bandwidth with later tensors. Warm-up matmuls hold the PE
            # p-state at max until the first projection inputs land.
            from collections import deque

            nc.sync.dma_start(wk_sb[:, :, :], wk_pk[:, :, :])
            nc.sync.dma_start(XK0[0][:, :, :], xk_pk[0, :, 0:4, :])
            nc.sync.dma_start(XK0[1][:, :, :], xk_pk[0, :, 4:8, :])
            nc.sync.dma_start(wq_sb[:, :, :], wq_pk[:, :, :])
            nc.sync.dma_start(XQ0[0][:, :, :], xq_pk[0, :, 0:4, :])
            nc.sync.dma_start(XQ0[1][:, :, :], xq_pk[0, :, 4:8, :])
            for m in range(2):
                for pj in range(2):
                    nc.sync.dma_start(b_sb[:, pj, m:m + 1],
                                      bqk[pj, m * 128:(m + 1) * 128, None])
            nc.sync.dma_start(wv_sb[:, :, :], wv_pk[:, :, :])

            heater(19, "w")  # PE busy from ~6.3us while the DMAs land

            # all sb0 projections first (their inputs are the DMA prefix);
            # then the remaining inputs continue on the same sync queue in
            # need-order: xv0, xk1, xv1, xk2, xk3, then Q blocks and wo
            emit_qk_proj(0, 1, [0])
            heater(4, "s1")
            emit_qk_proj(1, 1, [0])
            heater(5, "s2")
            emit_qk_proj(0, 0, [0])
            heater(3, "s3")
            emit_qk_proj(1, 0, [0])
            v_dma_filler(0, nc.sync)()
            nc.sync.dma_start(XK[1][:, :, :], xk_pk[1, :, :, :])
            v_dma_filler(1, nc.sync)()
            nc.sync.dma_start(XK[2][:, :, :], xk_pk[2, :, :, :])
            nc.sync.dma_start(XQ[1][:, :, :], xq_pk[1, :, :, :])
            nc.sync.dma_start(XK[3][:, :, :], xk_pk[3, :, :, :])
            nc.sync.dma_start(XQ[2][:, :, :], xq_pk[2, :, :, :])
            nc.sync.dma_start(XQ[3][:, :, :], xq_pk[3, :, :, :])
            nc.sync.dma_start(wo_sb[:, :, :], wo_pk[:, :, :])
            nc.sync.dma_start(wo2_sb[:, :], wo_pk[64:128, 1, :])

            # A1 (m=1 sb1-3 projections) dripped into later kt loops: K
            # parts during C0's qb1-3 (hp1-qb0's scores need all of KT[1]),
            # Q parts inside hp1-qb0's otherwise-empty schedule
            a1 = deque()
            for pj in (1, 0):
                for sb in range(1, 4):
                    a1.extend(qk_proj_fillers(1, pj, sb))

            # qb0 of C0 streams against its producers: K seq-blocks, V
            # tiles and their DMAs are scheduled so each lands with margin
            # before the lookahead score / PV that consumes it
            k1 = qk_proj_fillers(0, 1, 1)
            k2 = qk_proj_fillers(0, 1, 2)
            k3 = qk_proj_fillers(0, 1, 3)
            sched0 = {
                -2: [v_st_filler(0), k1[0]],
                -1: [v_st_filler(1), k1[1]],
                0: [v_st_filler(2)],
                1: [v_dma_filler(2), k2[0], v_st_filler(3)],
                2: [k2[1], v_st_filler(4)],
                3: [v_st_filler(5)],
                4: [v_st_filler(6), v_dma_filler(3), k3[0]],
                5: [k3[1], v_st_filler(7)],
            }
            for st in range(8, 16):
                sched0[st - 1] = [v_st_filler(st)]
            q1 = qk_proj_fillers(0, 0, 1)
            sched0.setdefault(12, []).append(q1[0])
            sched0.setdefault(13, []).append(q1[1])
            store0 = None
            if V_PRE:
                store0, pf0 = make_pre(0, 1)
                sched0.setdefault(14, []).append(pf0[0])
                sched0.setdefault(15, []).append(pf0[1])
            pa0 = alloc_pa(0, 0)
            emit_attn_block(0, 0, pa0, sched0)
            _nxt0 = store0

            def drain_sched(dq_, n, start=0):
                sched = {}
                for it in range(start, 16):
                    if dq_ and len(sched) < n:
                        sched[it] = [dq_.popleft()]
                return sched

            # each block's norm (pure DVE/DMA/GPSIMD work) is deferred into
            # the NEXT block's iter-0 slot, past its score lookahead, so the
            # PE never sits at a block boundary with an empty pipeline
            pending = [(0, 0, pa0)]

            def defer_norm(sched):
                while pending:
                    hp_, qb_, pa_ = pending.pop()
                    store = {}
                    sched.setdefault(0, []).insert(0, norm_copies(pa_, store))
                    sched.setdefault(2, []).append(
                        norm_finish(hp_, qb_, store))
                return sched

            # drain only K m1 + Q m1 sb0 (10 fillers) during hp0; the 6
            # Q m1 sb1-3 parts are reserved for hp1-qb0
            nxt = {"s": _nxt0}
            for qb, n in ((1, 2), (2, 2), (3, 2)):
                if qb < 3:
                    # Q-projection for the NEXT block, so its pre-emitted
                    # kt 0/1 scores (cross-block lookahead) read valid QT
                    emit_qk_proj(0, 0, [qb + 1])
                pa = alloc_pa(0, qb)
                sched = defer_norm(drain_sched(a1, n))
                store = None
                if V_PRE:
                    if qb < 3:
                        store, pf = make_pre(0, qb + 1)
                    else:
                        store, pf = make_pre(1, 0)
                    sched.setdefault(14, []).append(pf[0])
                    sched.setdefault(15, []).append(pf[1])
                emit_attn_block(0, qb, pa, sched, pre=nxt.pop("s", None))
                nxt["s"] = store
                pending.append((0, qb, pa))

            # out-projection dripped into C1's qb1-3 kt loops, one q-block
            # of ATT behind the attention that produces it
            dq = deque()
            held = deque()
            for qb in range(4):
                pa = alloc_pa(1, qb)
                # out-proj fillers sit in the back half: their ATT inputs
                # come from the norm deferred into this block's iter 0.
                # qb0 has no out-proj yet -- it runs the reserved Q m1 parts.
                # The last block drains qb2's fillers COMPLETELY and keeps
                # all exps on ScalarE (idle by then) so only its own 8
                # out-projs trail the final norm; heater matmuls bridge the
                # norm's reciprocal latency at max p-state.
                if qb == 0:
                    sched = drain_sched(a1, 4)
                    for it in (14, 15):
                        if a1:
                            sched.setdefault(it, []).append(a1.popleft())
                else:
                    # two fillers from TWO blocks ago open this block (their
                    # ATT has long been normalized, so they legally fill the
                    # iter -2/-1 slots where the post-lookahead score
                    # otherwise stalls ~1us on exp(0)'s PSUM recycle);
                    # six fillers from the previous block fill iters 8-13
                    sched = drain_sched(dq, 6, start=8)
                    for it in (-2, -1):
                        if held:
                            sched.setdefault(it, []).append(held.popleft())
                    while len(held) < 2 and dq:
                        held.append(dq.popleft())
                sched = defer_norm(sched)
                store = None
                if V_PRE and qb < 3:
                    store, pf = make_pre(1, qb + 1, dve=(qb + 1 != 3))
                    sched.setdefault(14, []).append(pf[0])
                    sched.setdefault(15, []).append(pf[1])
                emit_attn_block(1, qb, pa, sched,
                                dve=True, pre=nxt.pop("s", None))
                if store is not None:
                    nxt["s"] = store
                if qb == 3:
                    emit_attn_norm_fast(1, qb, pa,
                                        reserve=[lambda: heater(12, "t",
                                                                w=128)])
                else:
                    pending.append((1, qb, pa))
                for st in range(qb * 4, qb * 4 + 4):
                    for db in range(2):
                        dq.append(out_proj_filler(
                            st, db, tail=(qb == 3)))

            while held:
                held.popleft()()
            while dq:
                dq.popleft()()

    nc.compile()
    return nc


def _prep_inputs(q, k, v, wq, bq, wk, bk, wv, bv, wo, bo):
    q, k, v = (np.asarray(a, np.float32) for a in (q, k, v))
    wq, bq, wk, bk, wv, bv, wo, bo = (
        np.asarray(a, np.float32) for a in (wq, bq, wk, bk, wv, bv, wo, bo))

    def pack_x(x):  # [S, D] -> [4 sb, 128 p, 8 kt, 512 s] of x.T
        xT = np.ascontiguousarray(x.T)  # [D, S]
        return np.ascontiguousarray(
            xT.reshape(8, 128, 4, 512).transpose(2, 1, 0, 3)).astype(BF16)

    xP = {}
    for b in range(B):
        xP[("q", b)] = pack_x(q[b])
        xP[("k", b)] = pack_x(k[b])
        xP[("v", b)] = pack_x(v[b])

    def pack_w(wT):  # [D, DL] -> [128 p, 8 kt, DL]
        return np.ascontiguousarray(
            wT.reshape(8, 128, DL).transpose(1, 0, 2)).astype(BF16)

    grp = {}
    for g in range(GROUPS):
        hs = slice(g * DL, (g + 1) * DL)
        woT = np.ascontiguousarray(wo[:, hs].T)  # [DL, D]
        grp[g] = {
            "wq_pk": pack_w((wq[hs, :] * SCALE).T),
            "wk_pk": pack_w((wk[hs, :] * SCALE).T),
            "wv_pk": pack_w(wv[hs, :].T),
            "wo_pk": np.ascontiguousarray(
                woT.reshape(2, 128, D).transpose(1, 0, 2)).astype(BF16),
            "bqk": np.stack([bq[hs] * SCALE, bk[hs] * SCALE]).astype(np.float32),
        }

    in_maps = []
    for c in range(N_CORES):
        b, g = c // GROUPS, c % GROUPS
        m = {"xq_pk": xP[("q", b)], "xk_pk": xP[("k", b)],
             "xv_pk": xP[("v", b)]}
        m.update(grp[g])
        in_maps.append(m)

    const = (bv @ wo.T + bo).astype(np.float32)  # exact since sum(P) == 1
    return in_maps, const


def _run(in_maps, trace=False):
    from concourse.bass_utils import run_bass_kernel_spmd

    if "nc" not in _cache:
        _cache["nc"] = _build()
    return run_bass_kernel_spmd(_cache["nc"], in_maps, list(range(N_CORES)),
                                trace=trace)


def _reduce(results, const):
    out = np.zeros((B, S, D), np.float32)
    for c in range(N_CORES):
        out[c // GROUPS] += results[c]["out"].astype(np.float32)
    out += const
    return out


def kernel(**inputs) -> np.ndarray:
    in_maps, const = _prep_inputs(**inputs)
    res = _run(in_maps, trace=False)
    return _reduce(res.results, const)


def kernel_profiled(**inputs):
    """Returns (output, exec_time_ns or None)."""
    in_maps, const = _prep_inputs(**inputs)
    res = _run(in_maps, trace=True)
    return _reduce(res.results, const), res.exec_time_ns


# revision 45
# speedup vs baseline: 1.1943x; 1.1943x over previous
"""Multi-head attention (B=2, S=2048, D=1024, H=16) on 8 TRN2 NeuronCores.

Sharding: data-parallel over batch (2) x tensor-parallel over head groups (4).
Core c handles batch c//4, heads 4*(c%4) .. 4*(c%4)+3 (256 projection dims).
Each core computes its partial output projection; the host sums the 4 partials
per batch and adds the (bv @ wo.T + bo) constant, which is exact because
softmax weights sum to 1.

Device layouts (per core):
  QT/KT  [128, 2, 2048] bf16 : partition p + 128*m = local proj dim, free = seq
  V_sb   [128, 16, 4, 65] bf16: [k-pos within tile, k-tile, head, dk + ones col]
  scores computed transposed: ST[k, q] = K'h @ Q'h^T, exp mostly on ScalarE
  with a fraction offloaded to DVE via an int16 Schraudolph bit-trick,
  PV: attnT += [Vh|1]^T @ expST  (ones column yields the softmax denominator)
  out-proj: out[s, dout] = attnT^T @ woT, partial, bf16 to DRAM.

TRN2's tensor engine clocks down (2.4 -> 1.2 GHz) whenever it idles and needs
~3us of continuous execution to ramp back, so the emission is built around
never letting the PE queue drain: projections are dripped between attention
kt-steps, warm-up matmuls run while the first DMAs land, and heater matmuls
bridge the final normalization latency.
"""

import sys

sys.path.insert(0, "/opt/trn_rl_repo")

import numpy as np
import ml_dtypes

BF16 = ml_dtypes.bfloat16

B, S, D = 2, 2048, 1024
H, DK = 16, 64
N_CORES = 8
GROUPS = 4  # head groups (tensor-parallel)
DL = D // GROUPS  # 256 local projection dims per core
SCALE = 1.0 / np.sqrt(np.sqrt(float(DK)))  # fold 1/sqrt(dk) half into Q, half into K

# kt indices per (hp, qb) whose exp runs on DVE (Schraudolph) instead of ScalarE
DVE_KTS = (2, 5, 8, 11, 14)
# int16 Schraudolph constants: bits = s*128*log2(e) + 16256 + C (+0.5 trunc guard)
SCH_A = 184.66452
SCH_B = 16256.0 - 7.5 + 0.5

import os
V_HEAT = os.environ.get("V_HEAT", "1") == "1"
V_GPMUL = os.environ.get("V_GPMUL", "1") == "1"
V_PRE = os.environ.get("V_PRE", "0") == "1"
V_WO2 = os.environ.get("V_WO2", "0") == "1"

_cache: dict = {}


def _build():
    import concourse.mybir as mybir
    import concourse.tile as tile
    from concourse import bacc

    dt = mybir.dt
    f32, bf16 = dt.float32, dt.bfloat16

    nc = bacc.Bacc("TRN2", target_bir_lowering=False, debug=False,
                   num_devices=N_CORES)

    # prepacked inputs: [sb, p, kt, s] so each DMA is contiguous per partition
    xq_pk = nc.dram_tensor("xq_pk", [4, 128, 8, 512], bf16, kind="ExternalInput").ap()
    xk_pk = nc.dram_tensor("xk_pk", [4, 128, 8, 512], bf16, kind="ExternalInput").ap()
    xv_pk = nc.dram_tensor("xv_pk", [4, 128, 8, 512], bf16, kind="ExternalInput").ap()
    wq_pk = nc.dram_tensor("wq_pk", [128, 8, DL], bf16, kind="ExternalInput").ap()
    wk_pk = nc.dram_tensor("wk_pk", [128, 8, DL], bf16, kind="ExternalInput").ap()
    wv_pk = nc.dram_tensor("wv_pk", [128, 8, DL], bf16, kind="ExternalInput").ap()
    wo_pk = nc.dram_tensor("wo_pk", [128, 2, D], bf16, kind="ExternalInput").ap()
    bqk = nc.dram_tensor("bqk", [2, DL], f32, kind="ExternalInput").ap()
    out = nc.dram_tensor("out", [S, D], bf16, kind="ExternalOutput").ap()

    EXPF = mybir.ActivationFunctionType.Exp
    COPYF = mybir.ActivationFunctionType.Copy
    MULT, ADD = mybir.AluOpType.mult, mybir.AluOpType.add

    with tile.TileContext(nc) as tc:
        with (
            tc.tile_pool(name="res", bufs=1) as res,
            tc.tile_pool(name="wts", bufs=1) as wts,
            tc.tile_pool(name="xin", bufs=3) as xin,
            tc.tile_pool(name="expp", bufs=8) as expp,
            tc.tile_pool(name="nrm", bufs=4) as nrm,
            tc.tile_pool(name="drm", bufs=2, space="DRAM") as drm,
            tc.tile_pool(name="ps_proj", bufs=2, space="PSUM") as ps_proj,
            tc.tile_pool(name="ps_at", bufs=2, space="PSUM") as ps_at,
            tc.tile_pool(name="ps_st", bufs=2, space="PSUM") as ps_st,
        ):
            # ---- resident tensors ----
            QT = [res.tile([128, S], bf16, name=f"QT{m}", tag=f"QT{m}")
                  for m in range(2)]
            KT = [res.tile([128, S], bf16, name=f"KT{m}", tag=f"KT{m}")
                  for m in range(2)]
            Vsb = res.tile([128, 16, 4, DK + 1], bf16)
            ATT = [res.tile([128, S], bf16, name=f"ATT{m}", tag=f"ATT{m}")
                   for m in range(2)]

            wq_sb = wts.tile([128, 8, DL], bf16, tag="wq")
            wk_sb = wts.tile([128, 8, DL], bf16, tag="wk")
            wv_sb = wts.tile([128, 8, DL], bf16, tag="wv")
            wo_sb = wts.tile([128, 2, D], bf16, tag="wo")
            wo2_sb = wts.tile([64, D], bf16, tag="wo2")
            b_sb = wts.tile([128, 2, 2], f32, tag="b")  # [p, proj(q/k), m]
            ones_sb = wts.tile([1, 64], bf16, tag="ones")
            ones64f = wts.tile([65, 64], f32, tag="ones64")
            ones64b = wts.tile([65, 64], bf16, tag="ones64b")
            wrm_sb = wts.tile([1, 512], bf16, tag="wrm")

            # resident x tiles for Q/K; sb=0 is split in half so the first
            # projection's k-accumulation can start on the first 4 kt the
            # moment that half lands (fine-grained DMA deps)
            XQ = [None] + [res.tile([128, 8, 512], bf16, name=f"XQ{sb}",
                                    tag=f"XQ{sb}") for sb in range(1, 4)]
            XK = [None] + [res.tile([128, 8, 512], bf16, name=f"XK{sb}",
                                    tag=f"XK{sb}") for sb in range(1, 4)]
            XQ0 = [res.tile([128, 4, 512], bf16, name=f"XQ0{h}", tag=f"XQ0{h}")
                   for h in range(2)]
            XK0 = [res.tile([128, 4, 512], bf16, name=f"XK0{h}", tag=f"XK0{h}")
                   for h in range(2)]

            def xslice(pj, sb, kt):
                """rhs AP for (q/k) seq-block sb, contraction tile kt."""
                if sb == 0:
                    t = (XQ0 if pj == 0 else XK0)[kt // 4]
                    return t[:, kt % 4, :]
                t = (XQ if pj == 0 else XK)[sb]
                return t[:, kt, :]

            nc.vector.memset(Vsb[:, :, :, DK], 1.0)
            nc.vector.memset(ones_sb[:, :], 1.0)
            nc.vector.memset(ones64f[64:65, :], 1.0)
            nc.vector.memset(ones64b[64:65, :], 1.0)
            nc.vector.memset(wrm_sb[:, :], 1.0)

            # preload the Exp activation table (content unused)
            warm = nrm.tile([1, 1], f32, tag="warm")
            nc.scalar.activation(warm[:, :], wrm_sb[0:1, 0:1], EXPF)

            def heater(n, tag, w=512):
                """n dependency-free matmuls that keep the tensor engine's
                p-state at max through a gap (content discarded). w=128 gives
                ~53ns grains for fine-grained bridging."""
                if not V_HEAT:
                    return
                hps = ps_st.tile([128, 2, 512], f32, tag="st",
                                 name=f"heat{tag}")
                for i in range(n):
                    nc.tensor.matmul(hps[0:64, 0, 0:w], lhsT=ones_sb[:, :],
                                     rhs=wrm_sb[:, 0:w], start=True,
                                     stop=True)

            def qk_proj_fillers(m, pj, sb):
                """Two small pure-matmul closures (x is resident): first half
                of the k-accumulation, then second half + eviction. Dripped
                into attention's kt loop so they never hog PE long enough to
                starve ScalarE."""
                wsb, dst = [(wq_sb, QT), (wk_sb, KT)][pj]
                state = {}

                def part1():
                    ps = ps_proj.tile([128, 512], f32, tag="proj",
                                      name=f"psp{m}{pj}{sb}")
                    for kt in range(4):
                        nc.tensor.matmul(
                            ps[:, :],
                            lhsT=wsb[:, kt, m * 128:(m + 1) * 128],
                            rhs=xslice(pj, sb, kt),
                            start=(kt == 0), stop=False)
                    state["ps"] = ps

                def part2():
                    ps = state["ps"]
                    for kt in range(4, 8):
                        nc.tensor.matmul(
                            ps[:, :],
                            lhsT=wsb[:, kt, m * 128:(m + 1) * 128],
                            rhs=xslice(pj, sb, kt),
                            start=False, stop=(kt == 7))
                    nc.vector.tensor_scalar_add(
                        dst[m][:, sb * 512:(sb + 1) * 512],
                        ps[:, :], b_sb[:, pj, m:m + 1])

                return [part1, part2]

            def emit_qk_proj(m, pj, sbs):
                for sb in sbs:
                    for f in qk_proj_fillers(m, pj, sb):
                        f()

            vxb = {}

            def v_dma_filler(stq, eng=None):
                def f():
                    e = eng or nc.gpsimd
                    xb = xin.tile([128, 8, 512], bf16, tag="xblk",
                                  name=f"xbv{stq}")
                    e.dma_start(xb[:, 0:4, :], xv_pk[stq, :, 0:4, :])
                    e.dma_start(xb[:, 4:8, :], xv_pk[stq, :, 4:8, :])
                    vxb[stq] = xb
                return f

            def v_st_filler(st):
                def f():
                    xb = vxb[st // 4]
                    sts = st % 4
                    ps = ps_proj.tile([128, 4, DK], f32, tag="proj",
                                      name=f"psv{st}")
                    for kt in range(8):
                        nc.tensor.matmul(
                            ps[:, :, :],
                            lhsT=xb[:, kt, sts * 128:(sts + 1) * 128],
                            rhs=wv_sb[:, kt, :],
                            start=(kt == 0), stop=(kt == 7))
                    nc.vector.tensor_copy(Vsb[:, st, :, 0:DK], ps[:, :, :])
                return f

            def alloc_pa(hp, qb):
                return [ps_at.tile([65, 512], f32, tag="at",
                                   name=f"at{hp}{qb}{i}") for i in range(2)]

            def emit_attn_block(hp, qb, pa, sched=None, dve=True,
                                pre=None):
                """Software-pipelined 16-kt attention block: scores for kt+2
                are emitted before the exp-gated PV of kt so the in-order PE
                queue never idles behind a semaphore wait; drip fillers run
                between the lookahead scores and the PV. sched maps iter
                (-2..15) -> list of filler closures. pre carries exp tiles
                for kt 0/1 already emitted inside the PREVIOUS block's last
                iterations (cross-block lookahead), so this block's kt=2
                score never waits on exp(0) latency at the boundary."""
                qs = slice(qb * 512, (qb + 1) * 512)
                sched = sched or {}
                ex_tiles = pre if pre is not None else {}

                def sc_exp(kt):
                    # both heads' score tiles back-to-back: disjoint row
                    # groups (partitions 0-63 / 64-127) co-stream on PE
                    st_ps = ps_st.tile([128, 2, 512], f32, tag="st")
                    for hh in range(2):
                        lo, hi = hh * 64, hh * 64 + 64
                        nc.tensor.matmul(
                            st_ps[:, hh, :],
                            lhsT=KT[hp][lo:hi, kt * 128:(kt + 1) * 128],
                            rhs=QT[hp][lo:hi, qs],
                            start=True, stop=True)
                    ex = expp.tile([128, 2, 512], bf16, tag="exp")
                    if dve and kt in DVE_KTS:
                        # exp via int16 Schraudolph on DVE (result read as
                        # bf16); relieves the ScalarE exp bottleneck
                        nc.vector.tensor_scalar(
                            ex[:, :, :].bitcast(dt.int16), st_ps[:, :, :],
                            SCH_A, SCH_B, MULT, ADD)
                    else:
                        nc.scalar.activation(ex[:, :, :], st_ps[:, :, :], EXPF)
                    ex_tiles[kt] = ex

                def run(it):
                    for f in sched.get(it, ()):
                        f()

                run(-2)
                if 0 not in ex_tiles:
                    sc_exp(0)
                run(-1)
                if 1 not in ex_tiles:
                    sc_exp(1)
                # PV in kt-pairs: two consecutive matmuls of the SAME
                # accumulation group back-to-back overlap their stationary
                # loads with streaming (like the V-proj chains do), unlike
                # the alternating pa[0]/pa[1] order
                for kt2 in range(0, 16, 2):
                    if kt2 + 2 < 16:
                        sc_exp(kt2 + 2)
                    run(kt2)
                    if kt2 + 3 < 16:
                        sc_exp(kt2 + 3)
                    run(kt2 + 1)
                    exa = ex_tiles.pop(kt2)
                    exb = ex_tiles.pop(kt2 + 1)
                    for hh in range(2):
                        nc.tensor.matmul(
                            pa[hh][:, :],
                            lhsT=Vsb[:, kt2, 2 * hp + hh, :],
                            rhs=exa[:, hh, :],
                            start=(kt2 == 0), stop=False,
                            skip_group_check=True)
                        nc.tensor.matmul(
                            pa[hh][:, :],
                            lhsT=Vsb[:, kt2 + 1, 2 * hp + hh, :],
                            rhs=exb[:, hh, :],
                            start=False, stop=(kt2 + 1 == 15),
                            skip_group_check=True)

            def make_pre(hp, qb, dve=True):
                """Closures emitting block (hp,qb)'s kt 0/1 score+exp inside
                the previous block's sched[14]/[15] slots. The scores land in
                ps_proj tiles (free around block boundaries) rather than the
                2-deep ps_st ring, so the lookahead genuinely deepens instead
                of stealing the ring slots kt14/15 still need."""
                qs = slice(qb * 512, (qb + 1) * 512)
                store = {}

                def one(kt):
                    def f():
                        ph = [ps_proj.tile([128, 512], f32, tag="proj",
                                           name=f"pre{hp}{qb}{kt}{hh}")
                              for hh in range(2)]
                        for hh in range(2):
                            lo, hi = hh * 64, hh * 64 + 64
                            nc.tensor.matmul(
                                ph[hh][:, :],
                                lhsT=KT[hp][lo:hi, kt * 128:(kt + 1) * 128],
                                rhs=QT[hp][lo:hi, qs],
                                start=True, stop=True)
                        ex = expp.tile([128, 2, 512], bf16, tag="exp",
                                       name=f"prx{hp}{qb}{kt}")
                        for hh in range(2):
                            nc.scalar.activation(ex[:, hh, :], ph[hh][:, :],
                                                 EXPF)
                        store[kt] = ex
                    return f

                return store, [one(0), one(1)]

            def norm_copies(pa, store):
                """Phase 1 (next block's iter-0 slot): evict both attnT PSUM
                tiles to SBUF so the ps_at ring frees for the next block,
                and pre-cast the denominator rows so phase 2's broadcast
                matmuls fire without waiting on the DVE queue."""
                def f():
                    for hh in range(2):
                        asb = nrm.tile([65, 512], f32, tag="asb",
                                       name=f"asb{hh}")
                        nc.vector.tensor_copy(asb[:, :], pa[hh][:, :])
                        dn = nrm.tile([65, 512], bf16, tag="dn",
                                      name=f"dn{hh}")
                        with nc.allow_low_precision(reason="bf16 denom"):
                            nc.vector.tensor_copy(dn[64:65, :],
                                                  pa[hh][64:65, :])
                        store[hh] = asb
                        store[2 + hh] = dn
                return f

            def norm_finish(hp, qb, store):
                """Phase 2 (iter-2 slot, so its bcd PSUM ring slots clear
                before the drained out-proj fillers alloc): bf16-cast the
                denominator row, broadcast it down partitions 0..63 with a
                one-row matmul, invert with the fast approx reciprocal
                (base-0 only, hence broadcast-first), multiply on the idle
                GPSIMD engine."""
                def f():
                    qs = slice(qb * 512, (qb + 1) * 512)
                    rbs = {}
                    for hh in range(2):
                        dn = store[2 + hh]
                        bcd = ps_proj.tile([64, 512], f32, tag="proj",
                                           name=f"bcd{hp}{qb}{hh}")
                        nc.tensor.matmul(bcd[:, :], lhsT=ones64b[64:65, :],
                                         rhs=dn[64:65, :], start=True,
                                         stop=True)
                        rb = nrm.tile([64, 512], f32, tag="rbf",
                                      name=f"rb{hh}")
                        nc.vector.reciprocal_approx_fast(rb[:, :], bcd[:, :])
                        rbs[hh] = rb
                    eng = nc.gpsimd if V_GPMUL else nc.vector
                    eng.tensor_mul(ATT[hp][0:64, qs],
                                   store[0][0:64, :], rbs[0][:, :])
                    tmp = nrm.tile([64, 512], bf16, tag="tmp")
                    eng.tensor_mul(tmp[:, :], store[1][0:64, :], rbs[1][:, :])
                    nc.sync.dma_start(ATT[hp][64:128, qs], tmp[:, :])
                return f

            def emit_attn_norm_fast(hp, qb, pa, reserve=()):
                """Latency-lean endgame norm, fully serialized per head so
                hh1 (whose result must additionally hop to ATT partitions
                64..127 via DMA) completes first: bf16-cast denominator,
                one-row broadcast matmul, fast approx reciprocal (base-0
                only), multiply. Heaters keep the PE p-state at max through
                the DVE latency."""
                qs = slice(qb * 512, (qb + 1) * 512)
                for hh in (1, 0):
                    asb = nrm.tile([65, 512], f32, tag="asb",
                                   name=f"asbf{hh}")
                    nc.vector.tensor_copy(asb[:, :], pa[hh][:, :])
                    dn = nrm.tile([65, 512], bf16, tag="dn", name=f"dnf{hh}")
                    with nc.allow_low_precision(reason="bf16 denom"):
                        nc.vector.tensor_copy(dn[64:65, :], pa[hh][64:65, :])
                    if hh == 1:
                        for f in reserve:
                            f()
                    bcd = ps_proj.tile([64, 512], f32, tag="proj",
                                       name=f"bcdf{hh}")
                    nc.tensor.matmul(bcd[:, :], lhsT=ones64b[64:65, :],
                                     rhs=dn[64:65, :], start=True,
                                     stop=True)
                    rb = nrm.tile([64, 512], f32, tag="rbf",
                                  name=f"rbf{hh}")
                    nc.vector.reciprocal_approx_fast(rb[:, :], bcd[:, :])
                    if hh == 1:
                        tmp = nrm.tile([64, 512], bf16, tag="tmp")
                        nc.vector.tensor_mul(tmp[:, :], asb[0:64, :],
                                             rb[:, :])
                        endgame_tmp["t"] = tmp
                        if not V_WO2:
                            nc.sync.dma_start(ATT[hp][64:128, qs],
                                              tmp[:, :])
                            heater(18, "t2", w=128)
                        else:
                            heater(4, "t2", w=128)
                    else:
                        nc.vector.tensor_mul(ATT[hp][0:64, qs],
                                             asb[0:64, :], rb[:, :])

            endgame_tmp = {}

            def out_proj_filler(st, db, tail=False):
                def f():
                    ps = ps_proj.tile([128, 512], f32, tag="proj",
                                      name=f"pso{st}{db}")
                    dbs = slice(db * 512, (db + 1) * 512)
                    sts = slice(st * 128, (st + 1) * 128)
                    if tail and V_WO2:
                        # last block: hh1's normalized attn is read straight
                        # from the norm's tmp tile (base 0), so the ~2.5us
                        # SBUF-partition-hop DMA never gates these matmuls
                        tmp = endgame_tmp["t"]
                        qs2 = slice((st - 12) * 128, (st - 11) * 128)
                        nc.tensor.matmul(ps[:, :], lhsT=ATT[0][:, sts],
                                         rhs=wo_sb[:, 0, dbs],
                                         start=True, stop=False)
                        nc.tensor.matmul(ps[:, :], lhsT=ATT[1][0:64, sts],
                                         rhs=wo_sb[0:64, 1, dbs],
                                         start=False, stop=False)
                        nc.tensor.matmul(ps[:, :], lhsT=tmp[:, qs2],
                                         rhs=wo2_sb[:, dbs],
                                         start=False, stop=True)
                    else:
                        for m in range(2):
                            nc.tensor.matmul(
                                ps[:, :],
                                lhsT=ATT[m][:, sts],
                                rhs=wo_sb[:, m, dbs],
                                start=(m == 0), stop=(m == 1))
                    osb = nrm.tile([128, 512], bf16, tag="osb")
                    # at the endgame both ScalarE and DVE are idle: alternate
                    # so the final 8 casts don't serialize on one engine, and
                    # rotate the final DMAs across three DGE queues so their
                    # ~0.55us trigger setups don't serialize on the sync
                    # sequencer (that queueing was ~4us of pure tail)
                    if tail and (st + db) % 2 == 0:
                        nc.scalar.activation(osb[:, :], ps[:, :], COPYF)
                    else:
                        nc.vector.tensor_copy(osb[:, :], ps[:, :])
                    nc.sync.dma_start(
                        out[st * 128:(st + 1) * 128, db * 512:(db + 1) * 512],
                        osb[:, :])
                return f

            # ---- emission: flash-style streaming. K/V/Q chunks feed
            # attention's k-tile pipeline incrementally; A1 and the output
            # projection gap-fill PE while ScalarE (exp) runs flat out ----
            # The input phase is pure HBM-bandwidth-bound (~13.5MB), so all
            # transfers are issued on ONE queue in exact need-order: the
            # critical prefix (wk/xk0/wq/xq0/biases/wv) never shares
            # bandwidth with later tensors. Warm-up matmuls hold the PE
            # p-state at max until the first projection inputs land.
            from collections import deque

            nc.sync.dma_start(wk_sb[:, :, :], wk_pk[:, :, :])
            nc.sync.dma_start(XK0[0][:, :, :], xk_pk[0, :, 0:4, :])
            nc.sync.dma_start(XK0[1][:, :, :], xk_pk[0, :, 4:8, :])
            nc.sync.dma_start(wq_sb[:, :, :], wq_pk[:, :, :])
            nc.sync.dma_start(XQ0[0][:, :, :], xq_pk[0, :, 0:4, :])
            nc.sync.dma_start(XQ0[1][:, :, :], xq_pk[0, :, 4:8, :])
            for m in range(2):
                for pj in range(2):
                    nc.sync.dma_start(b_sb[:, pj, m:m + 1],
                                      bqk[pj, m * 128:(m + 1) * 128, None])
            nc.sync.dma_start(wv_sb[:, :, :], wv_pk[:, :, :])

            heater(19, "w")  # PE busy from ~6.3us while the DMAs land

            # all sb0 projections first (their inputs are the DMA prefix);
            # then the remaining inputs continue on the same sync queue in
            # need-order: xv0, xk1, xv1, xk2, xk3, then Q blocks and wo
            emit_qk_proj(0, 1, [0])
            heater(4, "s1")
            emit_qk_proj(1, 1, [0])
            heater(5, "s2")
            emit_qk_proj(0, 0, [0])
            heater(3, "s3")
            emit_qk_proj(1, 0, [0])
            v_dma_filler(0, nc.sync)()
            nc.sync.dma_start(XK[1][:, :, :], xk_pk[1, :, :, :])
            v_dma_filler(1, nc.sync)()
            nc.sync.dma_start(XK[2][:, :, :], xk_pk[2, :, :, :])
            nc.sync.dma_start(XQ[1][:, :, :], xq_pk[1, :, :, :])
            nc.sync.dma_start(XK[3][:, :, :], xk_pk[3, :, :, :])
            nc.sync.dma_start(XQ[2][:, :, :], xq_pk[2, :, :, :])
            nc.sync.dma_start(XQ[3][:, :, :], xq_pk[3, :, :, :])
            nc.sync.dma_start(wo_sb[:, :, :], wo_pk[:, :, :])
            nc.sync.dma_start(wo2_sb[:, :], wo_pk[64:128, 1, :])

            # A1 (m=1 sb1-3 projections) dripped into later kt loops: K
            # parts during C0's qb1-3 (hp1-qb0's scores need all of KT[1]),
            # Q parts inside hp1-qb0's otherwise-empty schedule
            a1 = deque()
            for pj in (1, 0):
                for sb in range(1, 4):
                    a1.extend(qk_proj_fillers(1, pj, sb))

            # qb0 of C0 streams against its producers: K seq-blocks, V
            # tiles and their DMAs are scheduled so each lands with margin
            # before the lookahead score / PV that consumes it
            k1 = qk_proj_fillers(0, 1, 1)
            k2 = qk_proj_fillers(0, 1, 2)
            k3 = qk_proj_fillers(0, 1, 3)
            sched0 = {
                -2: [v_st_filler(0), k1[0]],
                -1: [v_st_filler(1), k1[1]],
                0: [v_st_filler(2)],
                1: [v_dma_filler(2), k2[0], v_st_filler(3)],
                2: [k2[1], v_st_filler(4)],
                3: [v_st_filler(5)],
                4: [v_st_filler(6), v_dma_filler(3), k3[0]],
                5: [k3[1], v_st_filler(7)],
            }
            for st in range(8, 16):
                sched0[st - 1] = [v_st_filler(st)]
            q1 = qk_proj_fillers(0, 0, 1)
            sched0.setdefault(12, []).append(q1[0])
            sched0.setdefault(13, []).append(q1[1])
            store0 = None
            if V_PRE:
                store0, pf0 = make_pre(0, 1)
                sched0.setdefault(14, []).append(pf0[0])
                sched0.setdefault(15, []).append(pf0[1])
            pa0 = alloc_pa(0, 0)
            emit_attn_block(0, 0, pa0, sched0)
            _nxt0 = store0

            def drain_sched(dq_, n, start=0):
                sched = {}
                for it in range(start, 16):
                    if dq_ and len(sched) < n:
                        sched[it] = [dq_.popleft()]
                return sched

            # each block's norm (pure DVE/DMA/GPSIMD work) is deferred into
            # the NEXT block's iter-0 slot, past its score lookahead, so the
            # PE never sits at a block boundary with an empty pipeline
            pending = [(0, 0, pa0)]

            def defer_norm(sched):
                while pending:
                    hp_, qb_, pa_ = pending.pop()
                    store = {}
                    sched.setdefault(0, []).insert(0, norm_copies(pa_, store))
                    sched.setdefault(2, []).append(
                        norm_finish(hp_, qb_, store))
                return sched

            # drain only K m1 + Q m1 sb0 (10 fillers) during hp0; the 6
            # Q m1 sb1-3 parts are reserved for hp1-qb0
            nxt = {"s": _nxt0}
            for qb, n in ((1, 2), (2, 2), (3, 2)):
                if qb < 3:
                    # Q-projection for the NEXT block, so its pre-emitted
                    # kt 0/1 scores (cross-block lookahead) read valid QT
                    emit_qk_proj(0, 0, [qb + 1])
                pa = alloc_pa(0, qb)
                sched = defer_norm(drain_sched(a1, n))
                store = None
                if V_PRE:
                    if qb < 3:
                        store, pf = make_pre(0, qb + 1)
                    else:
                        store, pf = make_pre(1, 0)
                    sched.setdefault(14, []).append(pf[0])
                    sched.setdefault(15, []).append(pf[1])
                emit_attn_block(0, qb, pa, sched, pre=nxt.pop("s", None))
                nxt["s"] = store
                pending.append((0, qb, pa))

            # out-projection dripped into C1's qb1-3 kt loops, one q-block
            # of ATT behind the attention that produces it
            dq = deque()
            held = deque()
            for qb in range(4):
                pa = alloc_pa(1, qb)
                # out-proj fillers sit in the back half: their ATT inputs
                # come from the norm deferred into this block's iter 0.
                # qb0 has no out-proj yet -- it runs the reserved Q m1 parts.
                # The last block drains qb2's fillers COMPLETELY and keeps
                # all exps on ScalarE (idle by then) so only its own 8
                # out-projs trail the final norm; heater matmuls bridge the
                # norm's reciprocal latency at max p-state.
                if qb == 0:
                    sched = drain_sched(a1, 4)
                    for it in (14, 15):
                        if a1:
                            sched.setdefault(it, []).append(a1.popleft())
                else:
                    sched = drain_sched(dq, 8, start=8)
                sched = defer_norm(sched)
                store = None
                if V_PRE and qb < 3:
                    store, pf = make_pre(1, qb + 1, dve=(qb + 1 != 3))
                    sched.setdefault(14, []).append(pf[0])
                    sched.setdefault(15, []).append(pf[1])
                emit_attn_block(1, qb, pa, sched,
                                dve=True, pre=nxt.pop("s", None))
                if store is not None:
                    nxt["s"] = store
                if qb == 3:
                    emit_attn_norm_fast(1, qb, pa,
                                        reserve=[lambda: heater(12, "t",
                                                                w=128)])
                else:
                    pending.append((1, qb, pa))
                for st in range(qb * 4, qb * 4 + 4):
                    for db in range(2):
                        dq.append(out_proj_filler(
                            st, db, tail=(qb == 3)))

            while dq:
                dq.popleft()()

    nc.compile()
    return nc


def _prep_inputs(q, k, v, wq, bq, wk, bk, wv, bv, wo, bo):
    q, k, v = (np.asarray(a, np.float32) for a in (q, k, v))
    wq, bq, wk, bk, wv, bv, wo, bo = (
        np.asarray(a, np.float32) for a in (wq, bq, wk, bk, wv, bv, wo, bo))

    def pack_x(x):  # [S, D] -> [4 sb, 128 p, 8 kt, 512 s] of x.T
        xT = np.ascontiguousarray(x.T)  # [D, S]
        return np.ascontiguousarray(
            xT.reshape(8, 128, 4, 512).transpose(2, 1, 0, 3)).astype(BF16)

    xP = {}
    for b in range(B):
        xP[("q", b)] = pack_x(q[b])
        xP[("k", b)] = pack_x(k[b])
        xP[("v", b)] = pack_x(v[b])

    def pack_w(wT):  # [D, DL] -> [128 p, 8 kt, DL]
        return np.ascontiguousarray(
            wT.reshape(8, 128, DL).transpose(1, 0, 2)).astype(BF16)

    grp = {}
    for g in range(GROUPS):
        hs = slice(g * DL, (g + 1) * DL)
        woT = np.ascontiguousarray(wo[:, hs].T)  # [DL, D]
        grp[g] = {
            "wq_pk": pack_w((wq[hs, :] * SCALE).T),
            "wk_pk": pack_w((wk[hs, :] * SCALE).T),
            "wv_pk": pack_w(wv[hs, :].T),
            "wo_pk": np.ascontiguousarray(
                woT.reshape(2, 128, D).transpose(1, 0, 2)).astype(BF16),
            "bqk": np.stack([bq[hs] * SCALE, bk[hs] * SCALE]).astype(np.float32),
        }

    in_maps = []
    for c in range(N_CORES):
        b, g = c // GROUPS, c % GROUPS
        m = {"xq_pk": xP[("q", b)], "xk_pk": xP[("k", b)],
             "xv_pk": xP[("v", b)]}
        m.update(grp[g])
        in_maps.append(m)

    const = (bv @ wo.T + bo).astype(np.float32)  # exact since sum(P) == 1
    return in_maps, const


def _run(in_maps, trace=False):
    from concourse.bass_utils import run_bass_kernel_spmd

    if "nc" not in _cache:
        _cache["nc"] = _build()
    return run_bass_kernel_spmd(_cache["nc"], in_maps, list(range(N_CORES)),
                                trace=trace)


def _reduce(results, const):
    out = np.zeros((B, S, D), np.float32)
    for c in range(N_CORES):
        out[c // GROUPS] += results[c]["out"].astype(np.float32)
    out += const
    return out


def kernel(**inputs) -> np.ndarray:
    in_maps, const = _prep_inputs(**inputs)
    res = _run(in_maps, trace=False)
    return _reduce(res.results, const)


def kernel_profiled(**inputs):
    """Returns (output, exec_time_ns or None)."""
    in_maps, const = _prep_inputs(**inputs)
    res = _run(in_maps, trace=True)
    return _reduce(res.results, const), res.exec_time_ns
